# revision 4
# baseline (speedup 1.0000x reference)
"""Trainium2 Bass kernel for nn_AttentionResidualBlock.

Computation (per token t, head h):
    q = x @ W_q + b_q
    scores[t,h,l] = <q[t,h,:], k[t,l,h,:]> / sqrt(hd)   (k = layer_history)
    w = softmax_l(scores)
    out[t,h,:] = sum_l w[t,h,l] * k[t,l,h,:]

Sharding: data-parallel over the 8192 (b,s) tokens -> 8 cores x 1024 tokens.
Per-core layout: token-major (tokens on partitions), 8 tiles of 128 tokens.

Per tile:
  - layer_history arrives as bf16 via SWDGE cast-DMA (same HBM traffic,
    half the SBUF, and enables the DVE 2x_1P tensor_tensor mode)
  - q_proj on PE in fp32r (full rate at n=512, ~fp32 precision): the x tile
    is transposed with PE transposes, then 16 accumulating matmuls plus a
    k=1 "ones" matmul adds b_q; ACT copies PSUM->SBUF as bf16 with the
    1/sqrt(hd) scale folded in
  - scores: one DVE bf16 mul (q broadcast over l via a step-0 AP dim), then
    an in-place pairwise fold tree over hd (bf16 2x) with an fp32 tail
  - softmax over l=12 without max subtraction (scores ~ N(0,1))
  - normalized weights are written as bf16 pairs and broadcast across hd
    with step-0-source fp32-word copies on ACT, pipelined in 6 groups of 2
    layers with the weighted-sum muls so the serial ACT prefix stays short
  - weighted sum: DVE bf16 mul into a double-buffered product tile; the sum
    over l runs on PE as accumulating identity-matmul copies into PSUM
    (exact fp32 accumulation), with the PSUM->SBUF output drain deferred by
    one tile so ACT never stalls on the PE engine-counter semaphore
DVE is the bottleneck engine (~22 us/tile busy); DMA ~21 us/tile, PE ~17,
ACT ~11. Measured ~193 us/core on HW median (DMA roofline ~165 us).
"""

import math
from contextlib import ExitStack

import numpy as np

import concourse.tile as tile
from concourse import bacc, mybir
from concourse.bass_utils import run_bass_kernel_spmd
from concourse import masks

FP32 = mybir.dt.float32
FP32R = mybir.dt.float32r
BF16 = mybir.dt.bfloat16

B, S, L, D, H = 4, 2048, 12, 1024, 16
HD = D // H
N_CORES = 8
T = B * S // N_CORES          # tokens per core = 1024
P = 128                       # partition tile
NT = T // P                   # 8 token tiles per core
SCALE = 1.0 / math.sqrt(HD)   # 0.125


def build_body(ctx, tc, out, xt, kh, wq, bq, ones, repeat=1):
    nc = tc.nc

    const_pool = ctx.enter_context(tc.tile_pool(name="const", bufs=1))
    # W as lhsT chunks: w_sb[p, c, j] = W[c*128 + p, j]
    w_sb = const_pool.tile([P, 8, D], FP32R)
    wqr = wq.rearrange("(c p) j -> p c j", p=P).bitcast(FP32R)
    nc.scalar.dma_start(w_sb[:, :, 0:512], wqr[:, :, 0:512])
    nc.scalar.dma_start(w_sb[:, :, 512:1024], wqr[:, :, 512:1024])
    bq_sb = const_pool.tile([1, D], BF16)
    nc.gpsimd.dma_start(bq_sb[:], bq.unsqueeze(0))
    ones_sb = const_pool.tile([1, P], BF16)
    nc.gpsimd.dma_start(ones_sb[:], ones.unsqueeze(0))
    ident_bf = const_pool.tile([P, P], BF16)
    masks.make_identity(nc, ident_bf[:])

    kp = ctx.enter_context(tc.tile_pool(name="k", bufs=2))
    xtp = ctx.enter_context(tc.tile_pool(name="xt", bufs=2))
    qp = ctx.enter_context(tc.tile_pool(name="q", bufs=2))
    prodp = ctx.enter_context(tc.tile_pool(name="prod", bufs=1))
    p2p = ctx.enter_context(tc.tile_pool(name="p2", bufs=2))
    wbp = ctx.enter_context(tc.tile_pool(name="wb", bufs=1))
    sp = ctx.enter_context(tc.tile_pool(name="smx", bufs=2))
    ps_t = ctx.enter_context(tc.tile_pool(name="ps_t", bufs=2, space="PSUM"))
    ps_q = ctx.enter_context(tc.tile_pool(name="ps_q", bufs=1, space="PSUM"))
    ps_a = ctx.enter_context(tc.tile_pool(name="ps_a", bufs=2, space="PSUM"))

    # PE warm-up: ~4us of dummy matmuls at t~0 so the HAM clock-gate opens
    # before tile 0's q_proj (cold PE is the prologue critical path)
    warm_ps = ps_t.tile([P, P], FP32, tag="xtps")
    for i in range(32):
        nc.tensor.matmul(
            warm_ps[:], lhsT=ident_bf[:], rhs=ident_bf[:],
            start=(i == 0), stop=(i == 31),
        )

    def flush_pending(pending):
        # one-tile-deferred output drain: by now the PE sum-over-l matmuls
        # for that tile are long done, so ACT never stalls on the PE counter
        a_prev, tok_prev = pending
        o_sb = xtp.tile([P, D], FP32, tag="xt")
        nc.scalar.copy(o_sb[:], a_prev[:])
        nc.sync.dma_start(out[tok_prev], o_sb[:])

    pending = None
    for tt in range(NT * repeat):
        tt = tt % NT
        tok = slice(tt * P, (tt + 1) * P)

        # ---- loads ----
        k_bf = kp.tile([P, L, D], BF16, tag="k")
        if tt == 0:
            # split the first k load so tile 0's scores can start earlier
            nc.gpsimd.dma_start(k_bf[:, 0:6, :], kh[tok, 0:6, :])
            nc.gpsimd.dma_start(k_bf[:, 6:12, :], kh[tok, 6:12, :])
        else:
            nc.gpsimd.dma_start(k_bf[:], kh[tok])  # fp32 -> bf16 cast DMA
        # The whole q-production chain runs at high priority so the PE order
        # places it ahead of the previous tile's sum-over-l matmuls (the
        # scores-mul waits on q through engine-counter semaphores).
        with tc.high_priority(offset=180):
            # x arrives pre-transposed from the host: xt_sb[p, c, t]
            xt_sb = xtp.tile([P, 8, P], FP32R, tag="xt")
            nc.sync.dma_start(
                xt_sb[:],
                xt[:, tok].rearrange("(c p) t -> p c t", p=P).bitcast(FP32R),
            )

            # q = x @ W + b (token-major PSUM [t, d_out])
            q_ps = ps_q.tile([P, D], FP32, tag="qps")
            for half in range(2):
                n0 = half * 512
                for c in range(8):
                    nc.tensor.matmul(
                        q_ps[:, n0:n0 + 512],
                        lhsT=xt_sb[:, c, :],
                        rhs=w_sb[:, c, n0:n0 + 512],
                        start=(c == 0),
                        stop=False,
                    )
                nc.tensor.matmul(
                    q_ps[:, n0:n0 + 512],
                    lhsT=ones_sb[:],
                    rhs=bq_sb[:, n0:n0 + 512],
                    start=False,
                    stop=True,
                )
            # q -> SBUF bf16, folding in 1/sqrt(hd), on ACT (keeps DVE free
            # for the scores/weighted-sum muls, which are the busy engine)
            q_bf = qp.tile([P, D], BF16, tag="q")
            nc.scalar.mul(q_bf[:], q_ps[:], SCALE)

        if pending is not None:
            flush_pending(pending)
            pending = None

        # ---- scores: prod = k * q (broadcast over l), fold-reduce over hd ----
        k4 = k_bf[:].rearrange("p l (h e) -> p l h e", h=H)
        qv = (
            q_bf[:]
            .rearrange("p (h e) -> p h e", h=H)
            .unsqueeze(1)
            .broadcast_to([P, L, H, HD])
        )
        prod = prodp.tile([P, L, H, HD], BF16, tag="prod")
        scr = sp.tile([P, L, H], FP32, tag="scr")
        with tc.high_priority(offset=60):
            # tile 0: two l-halves so compute starts as soon as half of k is in
            for ls in ([slice(0, 6), slice(6, 12)] if tt == 0 else [slice(0, L)]):
                nl = ls.stop - ls.start
                nc.vector.tensor_mul(prod[:, ls], k4[:, ls], qv[:, ls])
                # in-place fold tree over hd: 64->32->...->2, then fp32 tail add.
                # dst aliases in1 exactly (same element positions) which is safe
                # for the streaming DVE.
                off = 0
                for w0 in (32, 16, 8, 4, 2):
                    nc.vector.tensor_add(
                        prod[:, ls, :, off + w0:off + 2 * w0],
                        prod[:, ls, :, off:off + w0],
                        prod[:, ls, :, off + w0:off + 2 * w0],
                    )
                    off += w0
                # off == 62: two surviving partials at 62, 63
                nc.vector.tensor_add(
                    scr[:, ls].unsqueeze(3),
                    prod[:, ls, :, 62:63],
                    prod[:, ls, :, 63:64],
                )


        # ---- softmax over l (no max subtraction) ----
        es = sp.tile([P, L, H], FP32, tag="es")
        nc.scalar.activation(es[:], scr[:], mybir.ActivationFunctionType.Exp)
        den = sp.tile([P, H], FP32, tag="den")
        nc.vector.tensor_reduce(
            den[:],
            es[:].rearrange("p l h -> p h l"),
            axis=mybir.AxisListType.X,
            op=mybir.AluOpType.add,
        )
        rd = sp.tile([P, H], FP32, tag="rd")
        nc.vector.reciprocal(rd[:], den[:])

        # normalized weights as bf16 pairs (a pair = one aligned fp32 word);
        # the weighted-sum mul below reads each pair 32x via a step-0 word
        # dim, so no expansion pass is needed.
        wb = wbp.tile([P, L, H, 2], BF16, tag="wb")
        rdv = rd[:].unsqueeze(1).broadcast_to([P, L, H]).unsqueeze(3)
        nc.vector.tensor_mul(
            wb[:],
            es[:].unsqueeze(3).broadcast_to([P, L, H, 2]),
            rdv.broadcast_to([P, L, H, 2]),
        )
        prod2 = p2p.tile([P, L, D], BF16, tag="p2")
        k5 = k_bf[:].rearrange("p l (h w e) -> p l h w e", h=H, w=HD // 2)
        p5 = prod2[:].rearrange("p l (h w e) -> p l h w e", h=H, w=HD // 2)
        acc = ps_a.tile([P, D], FP32, tag="acc")
        for lh in range(6):
            ls = slice(lh * 2, (lh + 1) * 2)
            # pair-broadcast: last dim [stride 1, count 2] keeps the DVE
            # 2x mode; the w dim walks the same fp32 word 32 times.
            wv = wb[:, ls].unsqueeze(3).broadcast_to([P, 2, H, HD // 2, 2])
            nc.vector.tensor_mul(p5[:, ls], k5[:, ls], wv)
            # sum over l on PE: accumulating identity-matmul copies (fp32
            # PSUM accumulation). prod2 is double-buffered so the DVE never
            # waits on these within a tile.
            for l in range(ls.start, ls.stop):
                for half in range(2):
                    n0 = half * 512
                    nc.tensor.matmul(
                        acc[:, n0:n0 + 512],
                        lhsT=ident_bf[:],
                        rhs=prod2[:, l, n0:n0 + 512],
                        start=(l == 0),
                        stop=(l == L - 1),
                    )
        pending = (acc, tok)

    flush_pending(pending)


_NC_CACHE = {}


def build_nc(repeat=1):
    if repeat in _NC_CACHE:
        return _NC_CACHE[repeat]
    nc = bacc.Bacc("TRN2", target_bir_lowering=False, debug=False,
                   num_devices=N_CORES)
    xt = nc.dram_tensor("xt", [D, T], FP32, kind="ExternalInput").ap()
    kh = nc.dram_tensor("kh", [T, L, D], FP32, kind="ExternalInput").ap()
    wq = nc.dram_tensor("wq", [D, D], FP32, kind="ExternalInput").ap()
    bq = nc.dram_tensor("bq", [D], FP32, kind="ExternalInput").ap()
    ones = nc.dram_tensor("ones", [P], FP32, kind="ExternalInput").ap()
    out = nc.dram_tensor("out", [T, D], FP32, kind="ExternalOutput").ap()
    with tile.TileContext(nc) as tc, ExitStack() as ctx:
        build_body(ctx, tc, out, xt, kh, wq, bq, ones, repeat=repeat)
    nc.compile()
    _NC_CACHE[repeat] = nc
    return nc


def make_in_maps(x_current, layer_history, W_q, b_q):
    x_flat = np.ascontiguousarray(
        x_current.reshape(B * S, D), dtype=np.float32)
    k_flat = np.ascontiguousarray(
        layer_history.reshape(B * S, L, D), dtype=np.float32)
    W_q = np.ascontiguousarray(W_q, dtype=np.float32)
    b_q = np.ascontiguousarray(b_q, dtype=np.float32)
    in_maps = []
    for c in range(N_CORES):
        sl = slice(c * T, (c + 1) * T)
        in_maps.append({
            "xt": np.ascontiguousarray(x_flat[sl].T),
            "kh": k_flat[sl],
            "wq": W_q,
            "bq": b_q,
            "ones": np.ones((P,), np.float32),
        })
    return in_maps


def kernel(x_current, layer_history, W_q, b_q):
    nc = build_nc()
    in_maps = make_in_maps(x_current, layer_history, W_q, b_q)
    res = run_bass_kernel_spmd(nc, in_maps, core_ids=list(range(N_CORES)))
    out = np.concatenate([res.results[c]["out"] for c in range(N_CORES)], axis=0)
    return out.reshape(B, S, D).astype(np.float32)


if __name__ == "__main__":
    rng = np.random.default_rng(0)
    x = rng.standard_normal((B, S, D), dtype=np.float32)
    k = rng.standard_normal((B, S, L, D), dtype=np.float32)
    W = (rng.standard_normal((D, D), dtype=np.float32) / math.sqrt(D)).astype(np.float32)
    b = (rng.standard_normal((D,), dtype=np.float32) * 0.01).astype(np.float32)
    o = kernel(x, k, W, b)
    print("ok", o.shape, o.dtype, float(np.abs(o).mean()))



# revision 11
# speedup vs baseline: 1.2721x; 1.2721x over previous
"""Trainium2 Bass kernel for nn_AttentionResidualBlock.

Computation (per token t, head h):
    q = x @ W_q + b_q
    scores[t,h,l] = <q[t,h,:], k[t,l,h,:]> / sqrt(hd)   (k = layer_history)
    w = softmax_l(scores)
    out[t,h,:] = sum_l w[t,h,l] * k[t,l,h,:]

Sharding: data-parallel over the 8192 (b,s) tokens -> 8 cores x 1024 tokens.
Per-core layout: token-major (tokens on partitions), 8 tiles of 128 tokens.

Per tile:
  - layer_history arrives as bf16 via SWDGE cast-DMA (same HBM traffic,
    half the SBUF, and enables the DVE 2x_1P tensor_tensor mode)
  - q_proj on PE in fp32r (full rate at n=512, ~fp32 precision): the x tile
    is transposed with PE transposes, then 16 accumulating matmuls plus a
    k=1 "ones" matmul adds b_q; ACT copies PSUM->SBUF as bf16 with the
    1/sqrt(hd) scale folded in
  - scores: one DVE bf16 mul (q broadcast over l via a step-0 AP dim), then
    an in-place pairwise fold tree over hd (bf16 2x) with an fp32 tail
  - softmax over l=12 without max subtraction (scores ~ N(0,1))
  - normalized weights are written as bf16 pairs and broadcast across hd
    with step-0-source fp32-word copies on ACT, pipelined in 6 groups of 2
    layers with the weighted-sum muls so the serial ACT prefix stays short
  - weighted sum: DVE bf16 mul into a double-buffered product tile; the sum
    over l runs on PE as accumulating identity-matmul copies into PSUM
    (exact fp32 accumulation), with the PSUM->SBUF output drain deferred by
    one tile so ACT never stalls on the PE engine-counter semaphore
DVE is the bottleneck engine (~22 us/tile busy); DMA ~21 us/tile, PE ~17,
ACT ~11. Measured ~193 us/core on HW median (DMA roofline ~165 us).
"""

import math
from contextlib import ExitStack

import numpy as np

import concourse.tile as tile
from concourse import bacc, mybir
from concourse.bass_utils import run_bass_kernel_spmd
from concourse import masks

FP32 = mybir.dt.float32
FP32R = mybir.dt.float32r
BF16 = mybir.dt.bfloat16

B, S, L, D, H = 4, 2048, 12, 1024, 16
HD = D // H
N_CORES = 8
T = B * S // N_CORES          # tokens per core = 1024
P = 128                       # partition tile
NT = T // P                   # 8 token tiles per core
SCALE = 1.0 / math.sqrt(HD)   # 0.125


def build_body(ctx, tc, out, xt, kh, wq, bq, ones, repeat=1):
    nc = tc.nc

    const_pool = ctx.enter_context(tc.tile_pool(name="const", bufs=1))
    # W as lhsT chunks: w_sb[p, c, j] = W[c*128 + p, j]
    w_sb = const_pool.tile([P, 8, D], FP32R)
    wqr = wq.rearrange("(c p) j -> p c j", p=P).bitcast(FP32R)
    nc.scalar.dma_start(w_sb[:, :, 0:512], wqr[:, :, 0:512])
    nc.scalar.dma_start(w_sb[:, :, 512:1024], wqr[:, :, 512:1024])
    bq_sb = const_pool.tile([1, D], BF16)
    nc.gpsimd.dma_start(bq_sb[:], bq.unsqueeze(0))
    ones_sb = const_pool.tile([1, P], BF16)
    nc.gpsimd.dma_start(ones_sb[:], ones.unsqueeze(0))
    ident_bf = const_pool.tile([P, P], BF16)
    masks.make_identity(nc, ident_bf[:])

    kp = ctx.enter_context(tc.tile_pool(name="k", bufs=3))
    xtp = ctx.enter_context(tc.tile_pool(name="xt", bufs=2))
    qp = ctx.enter_context(tc.tile_pool(name="q", bufs=2))
    prodp = ctx.enter_context(tc.tile_pool(name="prod", bufs=1))
    p2p = ctx.enter_context(tc.tile_pool(name="p2", bufs=3))
    wbp = ctx.enter_context(tc.tile_pool(name="wb", bufs=1))
    op_pool = ctx.enter_context(tc.tile_pool(name="osb", bufs=2))
    sp = ctx.enter_context(tc.tile_pool(name="smx", bufs=2))
    ps_t = ctx.enter_context(tc.tile_pool(name="ps_t", bufs=2, space="PSUM"))
    ps_q = ctx.enter_context(tc.tile_pool(name="ps_q", bufs=1, space="PSUM"))
    ps_a = ctx.enter_context(tc.tile_pool(name="ps_a", bufs=2, space="PSUM"))

    # PE warm-up: ~4us of dummy matmuls at t~0 so the HAM clock-gate opens
    # before tile 0's q_proj (cold PE is the prologue critical path)
    warm_ps = ps_t.tile([P, P], FP32, tag="xtps")
    for i in range(32):
        nc.tensor.matmul(
            warm_ps[:], lhsT=ident_bf[:], rhs=ident_bf[:],
            start=(i == 0), stop=(i == 31),
        )

    def flush_pending(pending):
        # one-tile-deferred output drain: by now the PE sum-over-l matmuls
        # for that tile are long done, so ACT never stalls on the PE counter.
        # The weighted sum is over UNNORMALIZED exp weights; the softmax
        # 1/den is applied here on the (otherwise idle) Pool engine so the
        # per-tile DVE stream never waits on the den/recip chain.
        a_prev, rd_prev, tok_prev = pending
        o_sb = op_pool.tile([P, D], FP32, tag="o")
        nc.scalar.copy(o_sb[:], a_prev[:])
        o3 = o_sb[:].rearrange("p (h e) -> p h e", h=H)
        nc.gpsimd.tensor_mul(
            o3, o3, rd_prev[:].unsqueeze(2).broadcast_to([P, H, HD])
        )
        nc.sync.dma_start(out[tok_prev], o_sb[:])

    pending = None
    for tt in range(NT * repeat):
        tt = tt % NT
        tok = slice(tt * P, (tt + 1) * P)

        # ---- loads ----
        k_bf = kp.tile([P, L, D], BF16, tag="k")
        if tt == 0:
            # split the first k load so tile 0's scores can start earlier
            nc.gpsimd.dma_start(k_bf[:, 0:6, :], kh[tok, 0:6, :])
            nc.gpsimd.dma_start(k_bf[:, 6:12, :], kh[tok, 6:12, :])
        else:
            nc.gpsimd.dma_start(k_bf[:], kh[tok])  # fp32 -> bf16 cast DMA
        # The whole q-production chain runs at high priority so the PE order
        # places it ahead of the previous tile's sum-over-l matmuls (the
        # scores-mul waits on q through engine-counter semaphores).
        with tc.high_priority(offset=180):
            # x arrives pre-transposed from the host: xt_sb[p, c, t]
            xt_sb = xtp.tile([P, 8, P], FP32R, tag="xt")
            nc.sync.dma_start(
                xt_sb[:],
                xt[:, tok].rearrange("(c p) t -> p c t", p=P).bitcast(FP32R),
            )

            # q = x @ W + b (token-major PSUM [t, d_out])
            q_ps = ps_q.tile([P, D], FP32, tag="qps")
            for half in range(2):
                n0 = half * 512
                for c in range(8):
                    nc.tensor.matmul(
                        q_ps[:, n0:n0 + 512],
                        lhsT=xt_sb[:, c, :],
                        rhs=w_sb[:, c, n0:n0 + 512],
                        start=(c == 0),
                        stop=False,
                    )
                nc.tensor.matmul(
                    q_ps[:, n0:n0 + 512],
                    lhsT=ones_sb[:],
                    rhs=bq_sb[:, n0:n0 + 512],
                    start=False,
                    stop=True,
                )
            # q -> SBUF bf16, folding in 1/sqrt(hd), on ACT (keeps DVE free
            # for the scores/weighted-sum muls, which are the busy engine)
            q_bf = qp.tile([P, D], BF16, tag="q")
            nc.scalar.mul(q_bf[:], q_ps[:], SCALE)

        if pending is not None:
            flush_pending(pending)
            pending = None

        # ---- scores: prod = k * q (broadcast over l), fold-reduce over hd ----
        k4 = k_bf[:].rearrange("p l (h e) -> p l h e", h=H)
        qv = (
            q_bf[:]
            .rearrange("p (h e) -> p h e", h=H)
            .unsqueeze(1)
            .broadcast_to([P, L, H, HD])
        )
        prod = prodp.tile([P, L, H, HD], BF16, tag="prod")
        scr = sp.tile([P, L, H], FP32, tag="scr")
        with tc.high_priority(offset=60):
            # tile 0: two l-halves so compute starts as soon as half of k is in
            for ls in ([slice(0, 6), slice(6, 12)] if tt == 0 else [slice(0, L)]):
                nl = ls.stop - ls.start
                nc.vector.tensor_mul(prod[:, ls], k4[:, ls], qv[:, ls])
                # in-place fold tree over hd: 64->32->...->2, then fp32 tail add.
                # dst aliases in1 exactly (same element positions) which is safe
                # for the streaming DVE.
                off = 0
                for w0 in (32, 16, 8, 4, 2):
                    nc.vector.tensor_add(
                        prod[:, ls, :, off + w0:off + 2 * w0],
                        prod[:, ls, :, off:off + w0],
                        prod[:, ls, :, off + w0:off + 2 * w0],
                    )
                    off += w0
                # off == 62: two surviving partials at 62, 63
                nc.vector.tensor_add(
                    scr[:, ls].unsqueeze(3),
                    prod[:, ls, :, 62:63],
                    prod[:, ls, :, 63:64],
                )


        # ---- softmax over l (no max subtraction), unnormalized ----
        # exp goes straight to bf16 PAIRS (a pair = one aligned fp32 word)
        # in the expanded-weights tile; den/recip run on the side and only
        # feed the deferred Pool normalize, so the ACT expansion and the
        # DVE weighted-sum muls depend on nothing but this one exp.
        wb = wbp.tile([P, L, H, HD], BF16, tag="wb")
        nc.scalar.activation(
            wb[:, :, :, 0:2],
            scr[:].unsqueeze(3).broadcast_to([P, L, H, 2]),
            mybir.ActivationFunctionType.Exp,
        )
        den = sp.tile([P, H], FP32, tag="den")
        nc.vector.tensor_reduce(
            den[:],
            wb[:, :, :, 0].rearrange("p l h -> p h l"),
            axis=mybir.AxisListType.X,
            op=mybir.AluOpType.add,
        )
        rd = sp.tile([P, H], FP32, tag="rd")
        nc.vector.reciprocal(rd[:], den[:])

        # expand each bf16 pair across hd with a step-0-source fp32-word
        # broadcast copy on ACT, in 3 groups of 4 layers pipelined with the
        # weighted-sum muls (fewer, larger DVE ops).
        wbf = wb[:].bitcast(FP32)  # [P, L, H, 32] fp32 words (bf16 pairs)
        wbflat = wb[:].rearrange("p l h e -> p l (h e)")
        acc = ps_a.tile([P, D], FP32, tag="acc")
        for lh in range(6):
            ls = slice(lh * 2, (lh + 1) * 2)
            nc.scalar.copy(
                wbf[:, ls, :, 1:32],
                wbf[:, ls, :, 0:1].broadcast_to([P, 2, H, 31]),
            )
            prod2 = p2p.tile([P, 2, D], BF16, tag="p2")
            nc.vector.tensor_mul(
                prod2[:], k_bf[:, ls, :], wbflat[:, ls, :]
            )
            # sum over l on PE: accumulating identity-matmul copies (fp32
            # PSUM accumulation). prod2 is multi-buffered so the DVE never
            # waits on these within a tile.
            for l in range(ls.start, ls.stop):
                for half in range(2):
                    n0 = half * 512
                    nc.tensor.matmul(
                        acc[:, n0:n0 + 512],
                        lhsT=ident_bf[:],
                        rhs=prod2[:, l - ls.start, n0:n0 + 512],
                        start=(l == 0),
                        stop=(l == L - 1),
                    )
        pending = (acc, rd, tok)

    flush_pending(pending)


_NC_CACHE = {}


def build_nc(repeat=1):
    if repeat in _NC_CACHE:
        return _NC_CACHE[repeat]
    nc = bacc.Bacc("TRN2", target_bir_lowering=False, debug=False,
                   num_devices=N_CORES)
    xt = nc.dram_tensor("xt", [D, T], FP32, kind="ExternalInput").ap()
    kh = nc.dram_tensor("kh", [T, L, D], FP32, kind="ExternalInput").ap()
    wq = nc.dram_tensor("wq", [D, D], FP32, kind="ExternalInput").ap()
    bq = nc.dram_tensor("bq", [D], FP32, kind="ExternalInput").ap()
    ones = nc.dram_tensor("ones", [P], FP32, kind="ExternalInput").ap()
    out = nc.dram_tensor("out", [T, D], FP32, kind="ExternalOutput").ap()
    with tile.TileContext(nc) as tc, ExitStack() as ctx:
        build_body(ctx, tc, out, xt, kh, wq, bq, ones, repeat=repeat)
    nc.compile()
    _NC_CACHE[repeat] = nc
    return nc


def make_in_maps(x_current, layer_history, W_q, b_q):
    x_flat = np.ascontiguousarray(
        x_current.reshape(B * S, D), dtype=np.float32)
    k_flat = np.ascontiguousarray(
        layer_history.reshape(B * S, L, D), dtype=np.float32)
    W_q = np.ascontiguousarray(W_q, dtype=np.float32)
    b_q = np.ascontiguousarray(b_q, dtype=np.float32)
    in_maps = []
    for c in range(N_CORES):
        sl = slice(c * T, (c + 1) * T)
        in_maps.append({
            "xt": np.ascontiguousarray(x_flat[sl].T),
            "kh": k_flat[sl],
            "wq": W_q,
            "bq": b_q,
            "ones": np.ones((P,), np.float32),
        })
    return in_maps


def kernel(x_current, layer_history, W_q, b_q):
    nc = build_nc()
    in_maps = make_in_maps(x_current, layer_history, W_q, b_q)
    res = run_bass_kernel_spmd(nc, in_maps, core_ids=list(range(N_CORES)))
    out = np.concatenate([res.results[c]["out"] for c in range(N_CORES)], axis=0)
    return out.reshape(B, S, D).astype(np.float32)


if __name__ == "__main__":
    rng = np.random.default_rng(0)
    x = rng.standard_normal((B, S, D), dtype=np.float32)
    k = rng.standard_normal((B, S, L, D), dtype=np.float32)
    W = (rng.standard_normal((D, D), dtype=np.float32) / math.sqrt(D)).astype(np.float32)
    b = (rng.standard_normal((D,), dtype=np.float32) * 0.01).astype(np.float32)
    o = kernel(x, k, W, b)
    print("ok", o.shape, o.dtype, float(np.abs(o).mean()))



# revision 26
# speedup vs baseline: 1.3747x; 1.0806x over previous
"""Trainium2 Bass kernel for nn_AttentionResidualBlock.

Computation (per token t, head h):
    q = x @ W_q + b_q
    scores[t,h,l] = <q[t,h,:], k[t,l,h,:]> / sqrt(hd)   (k = layer_history)
    w = softmax_l(scores)
    out[t,h,:] = sum_l w[t,h,l] * k[t,l,h,:]

Sharding: data-parallel over the 8192 (b,s) tokens -> 8 cores x 1024 tokens.
Per-core layout: token-major (tokens on partitions), 8 tiles of 128 tokens.

Per tile:
  - layer_history arrives as bf16 via SWDGE cast-DMA (same HBM traffic,
    half the SBUF, and enables the DVE 2x_1P tensor_tensor mode)
  - q_proj on PE in fp32r (full rate at n=512, ~fp32 precision): the x tile
    is transposed with PE transposes, then 16 accumulating matmuls plus a
    k=1 "ones" matmul adds b_q; ACT copies PSUM->SBUF as bf16 with the
    1/sqrt(hd) scale folded in
  - scores: one DVE bf16 mul (q broadcast over l via a step-0 AP dim), then
    an in-place pairwise fold tree over hd (bf16 2x) with an fp32 tail
  - softmax over l=12 without max subtraction (scores ~ N(0,1))
  - normalized weights are written as bf16 pairs and broadcast across hd
    with step-0-source fp32-word copies on ACT, pipelined in 6 groups of 2
    layers with the weighted-sum muls so the serial ACT prefix stays short
  - weighted sum: DVE bf16 mul into a double-buffered product tile; the sum
    over l runs on PE as accumulating identity-matmul copies into PSUM
    (exact fp32 accumulation), with the PSUM->SBUF output drain deferred by
    one tile so ACT never stalls on the PE engine-counter semaphore
DVE is the bottleneck engine (~22 us/tile busy); DMA ~21 us/tile, PE ~17,
ACT ~11. Measured ~193 us/core on HW median (DMA roofline ~165 us).
"""

import math
from contextlib import ExitStack

import numpy as np

import concourse.tile as tile
from concourse import bacc, mybir
from concourse.bass_utils import run_bass_kernel_spmd
from concourse import masks

FP32 = mybir.dt.float32
FP32R = mybir.dt.float32r
BF16 = mybir.dt.bfloat16

B, S, L, D, H = 4, 2048, 12, 1024, 16
HD = D // H
N_CORES = 8
T = B * S // N_CORES          # tokens per core = 1024
P = 128                       # partition tile
NT = T // P                   # 8 token tiles per core
SCALE = 1.0 / math.sqrt(HD)   # 0.125
SPLIT_K = False                # per-tile kh DMA + scores cascade in 2 halves


def build_body(ctx, tc, out, xt, kh, wq, bq, ones, repeat=1):
    nc = tc.nc

    const_pool = ctx.enter_context(tc.tile_pool(name="const", bufs=1))
    # W as lhsT chunks: w_sb[p, c, j] = W[c*128 + p, j]
    w_sb = const_pool.tile([P, 8, D], FP32R)
    wqr = wq.rearrange("(c p) j -> p c j", p=P).bitcast(FP32R)
    nc.scalar.dma_start(w_sb[:, :, 0:512], wqr[:, :, 0:512])
    nc.scalar.dma_start(w_sb[:, :, 512:1024], wqr[:, :, 512:1024])
    bq_sb = const_pool.tile([1, D], BF16)
    nc.gpsimd.dma_start(bq_sb[:], bq.unsqueeze(0))
    ones_sb = const_pool.tile([1, P], BF16)
    nc.gpsimd.dma_start(ones_sb[:], ones.unsqueeze(0))
    ident_bf = const_pool.tile([P, P], BF16)
    masks.make_identity(nc, ident_bf[:])

    kp = ctx.enter_context(tc.tile_pool(name="k", bufs=3))
    xtp = ctx.enter_context(tc.tile_pool(name="xt", bufs=2))
    qp = ctx.enter_context(tc.tile_pool(name="q", bufs=2))
    prodp = ctx.enter_context(tc.tile_pool(name="prod", bufs=1))
    p2p = ctx.enter_context(tc.tile_pool(name="p2", bufs=3))
    wbp = ctx.enter_context(tc.tile_pool(name="wb", bufs=1))
    op_pool = ctx.enter_context(tc.tile_pool(name="osb", bufs=2))
    sp = ctx.enter_context(tc.tile_pool(name="smx", bufs=2))
    ps_t = ctx.enter_context(tc.tile_pool(name="ps_t", bufs=2, space="PSUM"))
    ps_q = ctx.enter_context(tc.tile_pool(name="ps_q", bufs=1, space="PSUM"))
    ps_a = ctx.enter_context(tc.tile_pool(name="ps_a", bufs=2, space="PSUM"))

    # PE warm-up: ~4us of dummy matmuls at t~0 so the HAM clock-gate opens
    # before tile 0's q_proj (cold PE is the prologue critical path)
    warm_ps = ps_t.tile([P, P], FP32, tag="xtps")
    for i in range(32):
        nc.tensor.matmul(
            warm_ps[:], lhsT=ident_bf[:], rhs=ident_bf[:],
            start=(i == 0), stop=(i == 31),
        )

    def flush_pending(pending):
        # one-tile-deferred output drain: by now the PE sum-over-l matmuls
        # for that tile are long done, so ACT never stalls on the PE counter.
        # The weighted sum is over UNNORMALIZED exp weights; the softmax
        # 1/den is applied here on the (otherwise idle) Pool engine so the
        # per-tile DVE stream never waits on the den/recip chain.
        a_prev, rd_prev, tok_prev = pending
        o_sb = op_pool.tile([P, D], FP32, tag="o")
        nc.scalar.copy(o_sb[:], a_prev[:])
        o3 = o_sb[:].rearrange("p (h e) -> p h e", h=H)
        nc.gpsimd.tensor_mul(
            o3, o3, rd_prev[:].unsqueeze(2).broadcast_to([P, H, HD])
        )
        nc.gpsimd.dma_start(out[tok_prev], o_sb[:])

    # ---- software-pipelined loads, one iteration ahead ----
    # ALL DMAs go through the one SWDGE (Pool) ring so their bus order is
    # exactly program order: xt first (tiny; unblocks the PE q_proj that
    # overlaps the k stream), then the big kh cast-DMA, then the previous
    # tile's output store in the slack. Issuing tile i+1's loads at the TOP
    # of iteration i keeps the ring fed so the bus never idles waiting for
    # the Q7 to generate descriptors at an iteration boundary.
    n_iters = NT * repeat
    xt_tiles, k_tiles = {}, {}

    def load_tile(i):
        if i >= n_iters:
            return
        tok_i = slice((i % NT) * P, (i % NT + 1) * P)
        # x arrives pre-arranged from the host as [NT, p, c, t] so each tile
        # is 128 contiguous 4KB rows (one descriptor per partition)
        xs = xtp.tile([P, 8, P], FP32R, tag="xt")
        nc.gpsimd.dma_start(xs[:], xt[i % NT].bitcast(FP32R))
        # kh arrives as bf16 via SWDGE cast-DMA. With SPLIT_K, each tile's
        # load is two 6-layer halves so the scores/exp cascade can start
        # while the second half is still streaming in.
        kb = kp.tile([P, L, D], BF16, tag="k")
        if SPLIT_K or i == 0:
            nc.gpsimd.dma_start(kb[:, 0:6, :], kh[tok_i, 0:6, :])
            nc.gpsimd.dma_start(kb[:, 6:12, :], kh[tok_i, 6:12, :])
        else:
            nc.gpsimd.dma_start(kb[:], kh[tok_i])
        xt_tiles[i] = xs
        k_tiles[i] = kb

    pending = None
    load_tile(0)
    for i in range(n_iters):
        tt = i % NT
        tok = slice(tt * P, (tt + 1) * P)

        load_tile(i + 1)
        xt_sb = xt_tiles.pop(i)
        k_bf = k_tiles.pop(i)
        # The whole q-production chain runs at high priority so the PE order
        # places it ahead of the previous tile's sum-over-l matmuls (the
        # scores-mul waits on q through engine-counter semaphores).
        with tc.high_priority(offset=180):

            # q = x @ W + b (token-major PSUM [t, d_out])
            q_ps = ps_q.tile([P, D], FP32, tag="qps")
            for half in range(2):
                n0 = half * 512
                for c in range(8):
                    nc.tensor.matmul(
                        q_ps[:, n0:n0 + 512],
                        lhsT=xt_sb[:, c, :],
                        rhs=w_sb[:, c, n0:n0 + 512],
                        start=(c == 0),
                        stop=False,
                    )
                nc.tensor.matmul(
                    q_ps[:, n0:n0 + 512],
                    lhsT=ones_sb[:],
                    rhs=bq_sb[:, n0:n0 + 512],
                    start=False,
                    stop=True,
                )
            # q -> SBUF bf16, folding in 1/sqrt(hd), on ACT (keeps DVE free
            # for the scores/weighted-sum muls, which are the busy engine)
            q_bf = qp.tile([P, D], BF16, tag="q")
            nc.scalar.mul(q_bf[:], q_ps[:], SCALE)

        # ---- scores: prod = k * q (broadcast over l), fold-reduce over hd ----
        k4 = k_bf[:].rearrange("p l (h e) -> p l h e", h=H)
        qv = (
            q_bf[:]
            .rearrange("p (h e) -> p h e", h=H)
            .unsqueeze(1)
            .broadcast_to([P, L, H, HD])
        )
        prod = prodp.tile([P, L, H, HD], BF16, tag="prod")
        scr = sp.tile([P, L, H], FP32, tag="scr")
        # exp goes straight to bf16 PAIRS (a pair = one aligned fp32 word)
        # in the expanded-weights tile; den/recip run on the side and only
        # feed the deferred Pool normalize, so the ACT expansion and the
        # DVE weighted-sum muls depend on nothing but the per-half exp.
        wb = wbp.tile([P, L, H, HD], BF16, tag="wb")
        with tc.high_priority(offset=60):
            # two l-halves so compute starts as soon as half of k is in
            halves = (slice(0, 6), slice(6, 12)) if (SPLIT_K or tt == 0) \
                else (slice(0, L),)
            for ls in halves:
                nl = ls.stop - ls.start
                nc.vector.tensor_mul(prod[:, ls], k4[:, ls], qv[:, ls])
                # in-place fold tree over hd: 64->32->...->2, then fp32 tail add.
                # dst aliases in1 exactly (same element positions) which is safe
                # for the streaming DVE.
                off = 0
                for w0 in (32, 16, 8, 4, 2):
                    nc.vector.tensor_add(
                        prod[:, ls, :, off + w0:off + 2 * w0],
                        prod[:, ls, :, off:off + w0],
                        prod[:, ls, :, off + w0:off + 2 * w0],
                    )
                    off += w0
                # off == 62: two surviving partials at 62, 63
                nc.vector.tensor_add(
                    scr[:, ls].unsqueeze(3),
                    prod[:, ls, :, 62:63],
                    prod[:, ls, :, 63:64],
                )
                nc.scalar.activation(
                    wb[:, ls, :, 0:2],
                    scr[:, ls].unsqueeze(3).broadcast_to([P, nl, H, 2]),
                    mybir.ActivationFunctionType.Exp,
                )

        den = sp.tile([P, H], FP32, tag="den")
        nc.vector.tensor_reduce(
            den[:],
            wb[:, :, :, 0].rearrange("p l h -> p h l"),
            axis=mybir.AxisListType.X,
            op=mybir.AluOpType.add,
        )
        rd = sp.tile([P, H], FP32, tag="rd")
        nc.vector.reciprocal(rd[:], den[:])

        # expand each bf16 pair across hd with a step-0-source fp32-word
        # broadcast copy on ACT, in 3 groups of 4 layers pipelined with the
        # weighted-sum muls (fewer, larger DVE ops).
        wbf = wb[:].bitcast(FP32)  # [P, L, H, 32] fp32 words (bf16 pairs)
        wbflat = wb[:].rearrange("p l h e -> p l (h e)")
        acc = ps_a.tile([P, D], FP32, tag="acc")
        for lh in range(6):
            ls = slice(lh * 2, (lh + 1) * 2)
            nc.scalar.copy(
                wbf[:, ls, :, 1:32],
                wbf[:, ls, :, 0:1].broadcast_to([P, 2, H, 31]),
            )
            prod2 = p2p.tile([P, 2, D], BF16, tag="p2")
            nc.vector.tensor_mul(
                prod2[:], k_bf[:, ls, :], wbflat[:, ls, :]
            )
            # sum over l on PE: accumulating identity-matmul copies (fp32
            # PSUM accumulation). prod2 is multi-buffered so the DVE never
            # waits on these within a tile.
            for l in range(ls.start, ls.stop):
                for half in range(2):
                    n0 = half * 512
                    nc.tensor.matmul(
                        acc[:, n0:n0 + 512],
                        lhsT=ident_bf[:],
                        rhs=prod2[:, l - ls.start, n0:n0 + 512],
                        start=(l == 0),
                        stop=(l == L - 1),
                    )
        # flush the PREVIOUS tile here, at the bottom of the iteration: the
        # Pool normalize then runs while this tile's kh stream still owns the
        # DMA bus, instead of sitting between this tile's DMA triggers.
        if pending is not None:
            flush_pending(pending)
        pending = (acc, rd, tok)

    flush_pending(pending)


_NC_CACHE = {}


def build_nc(repeat=1):
    if repeat in _NC_CACHE:
        return _NC_CACHE[repeat]
    nc = bacc.Bacc("TRN2", target_bir_lowering=False, debug=False,
                   num_devices=N_CORES)
    xt = nc.dram_tensor("xt", [NT, P, 8, P], FP32, kind="ExternalInput").ap()
    kh = nc.dram_tensor("kh", [T, L, D], FP32, kind="ExternalInput").ap()
    wq = nc.dram_tensor("wq", [D, D], FP32, kind="ExternalInput").ap()
    bq = nc.dram_tensor("bq", [D], FP32, kind="ExternalInput").ap()
    ones = nc.dram_tensor("ones", [P], FP32, kind="ExternalInput").ap()
    out = nc.dram_tensor("out", [T, D], FP32, kind="ExternalOutput").ap()
    with tile.TileContext(nc) as tc, ExitStack() as ctx:
        build_body(ctx, tc, out, xt, kh, wq, bq, ones, repeat=repeat)
    nc.compile()
    _NC_CACHE[repeat] = nc
    return nc


def make_in_maps(x_current, layer_history, W_q, b_q):
    x_flat = np.ascontiguousarray(
        x_current.reshape(B * S, D), dtype=np.float32)
    k_flat = np.ascontiguousarray(
        layer_history.reshape(B * S, L, D), dtype=np.float32)
    W_q = np.ascontiguousarray(W_q, dtype=np.float32)
    b_q = np.ascontiguousarray(b_q, dtype=np.float32)
    in_maps = []
    for c in range(N_CORES):
        sl = slice(c * T, (c + 1) * T)
        # [NT, p, c, t]: xt[tt, p, ch, t] = x[tt*128 + t, ch*128 + p]
        xt_host = np.ascontiguousarray(
            x_flat[sl].reshape(NT, P, 8, P).transpose(0, 3, 2, 1))
        in_maps.append({
            "xt": xt_host,
            "kh": k_flat[sl],
            "wq": W_q,
            "bq": b_q,
            "ones": np.ones((P,), np.float32),
        })
    return in_maps


def kernel(x_current, layer_history, W_q, b_q):
    nc = build_nc()
    in_maps = make_in_maps(x_current, layer_history, W_q, b_q)
    res = run_bass_kernel_spmd(nc, in_maps, core_ids=list(range(N_CORES)))
    out = np.concatenate([res.results[c]["out"] for c in range(N_CORES)], axis=0)
    return out.reshape(B, S, D).astype(np.float32)


if __name__ == "__main__":
    rng = np.random.default_rng(0)
    x = rng.standard_normal((B, S, D), dtype=np.float32)
    k = rng.standard_normal((B, S, L, D), dtype=np.float32)
    W = (rng.standard_normal((D, D), dtype=np.float32) / math.sqrt(D)).astype(np.float32)
    b = (rng.standard_normal((D,), dtype=np.float32) * 0.01).astype(np.float32)
    o = kernel(x, k, W, b)
    print("ok", o.shape, o.dtype, float(np.abs(o).mean()))



# revision 27
# speedup vs baseline: 1.6551x; 1.2040x over previous
"""Trainium2 Bass kernel for nn_AttentionResidualBlock.

Computation (per token t, head h):
    q = x @ W_q + b_q
    scores[t,h,l] = <q[t,h,:], k[t,l,h,:]> / sqrt(hd)   (k = layer_history)
    w = softmax_l(scores)
    out[t,h,:] = sum_l w[t,h,l] * k[t,l,h,:]

Sharding: data-parallel over the 8192 (b,s) tokens -> 8 cores x 1024 tokens.
Per-core layout: token-major (tokens on partitions), 8 tiles of 128 tokens.

Per tile:
  - layer_history arrives as bf16 via SWDGE cast-DMA (same HBM traffic,
    half the SBUF, and enables the DVE 2x_1P tensor_tensor mode)
  - q_proj on PE in fp32r (full rate at n=512, ~fp32 precision): the x tile
    is transposed with PE transposes, then 16 accumulating matmuls plus a
    k=1 "ones" matmul adds b_q; ACT copies PSUM->SBUF as bf16 with the
    1/sqrt(hd) scale folded in
  - scores: one DVE bf16 mul (q broadcast over l via a step-0 AP dim), then
    an in-place pairwise fold tree over hd (bf16 2x) with an fp32 tail
  - softmax over l=12 without max subtraction (scores ~ N(0,1))
  - normalized weights are written as bf16 pairs and broadcast across hd
    with step-0-source fp32-word copies on ACT, pipelined in 6 groups of 2
    layers with the weighted-sum muls so the serial ACT prefix stays short
  - weighted sum: DVE bf16 mul into a double-buffered product tile; the sum
    over l runs on PE as accumulating identity-matmul copies into PSUM
    (exact fp32 accumulation), with the PSUM->SBUF output drain deferred by
    one tile so ACT never stalls on the PE engine-counter semaphore
DVE is the bottleneck engine (~22 us/tile busy); DMA ~21 us/tile, PE ~17,
ACT ~11. Measured ~193 us/core on HW median (DMA roofline ~165 us).
"""

import math
from contextlib import ExitStack

import numpy as np

import concourse.tile as tile
from concourse import bacc, mybir
from concourse.bass_utils import run_bass_kernel_spmd
from concourse import masks

FP32 = mybir.dt.float32
FP32R = mybir.dt.float32r
BF16 = mybir.dt.bfloat16

B, S, L, D, H = 4, 2048, 12, 1024, 16
HD = D // H
N_CORES = 8
T = B * S // N_CORES          # tokens per core = 1024
P = 128                       # partition tile
NT = T // P                   # 8 token tiles per core
SCALE = 1.0 / math.sqrt(HD)   # 0.125
SPLIT_K = False                # per-tile kh DMA + scores cascade in 2 halves


def build_body(ctx, tc, out, xt, kh, wq, bq, ones, repeat=1):
    nc = tc.nc

    const_pool = ctx.enter_context(tc.tile_pool(name="const", bufs=1))
    # W as lhsT chunks: w_sb[p, c, j] = W[c*128 + p, j]
    w_sb = const_pool.tile([P, 8, D], FP32R)
    wqr = wq.rearrange("(c p) j -> p c j", p=P).bitcast(FP32R)
    nc.scalar.dma_start(w_sb[:, :, 0:512], wqr[:, :, 0:512])
    nc.scalar.dma_start(w_sb[:, :, 512:1024], wqr[:, :, 512:1024])
    bq_sb = const_pool.tile([1, D], BF16)
    nc.gpsimd.dma_start(bq_sb[:], bq.unsqueeze(0))
    ones_sb = const_pool.tile([1, P], BF16)
    nc.gpsimd.dma_start(ones_sb[:], ones.unsqueeze(0))
    ident_bf = const_pool.tile([P, P], BF16)
    masks.make_identity(nc, ident_bf[:])

    kp = ctx.enter_context(tc.tile_pool(name="k", bufs=3))
    xtp = ctx.enter_context(tc.tile_pool(name="xt", bufs=3))
    qp = ctx.enter_context(tc.tile_pool(name="q", bufs=3))
    prodp = ctx.enter_context(tc.tile_pool(name="prod", bufs=1))
    p2p = ctx.enter_context(tc.tile_pool(name="p2", bufs=3))
    wbp = ctx.enter_context(tc.tile_pool(name="wb", bufs=1))
    op_pool = ctx.enter_context(tc.tile_pool(name="osb", bufs=2))
    sp = ctx.enter_context(tc.tile_pool(name="smx", bufs=2))
    ps_t = ctx.enter_context(tc.tile_pool(name="ps_t", bufs=2, space="PSUM"))
    ps_q = ctx.enter_context(tc.tile_pool(name="ps_q", bufs=1, space="PSUM"))
    ps_a = ctx.enter_context(tc.tile_pool(name="ps_a", bufs=2, space="PSUM"))

    # PE warm-up: ~4us of dummy matmuls at t~0 so the HAM clock-gate opens
    # before tile 0's q_proj (cold PE is the prologue critical path)
    warm_ps = ps_t.tile([P, P], FP32, tag="xtps")
    for i in range(32):
        nc.tensor.matmul(
            warm_ps[:], lhsT=ident_bf[:], rhs=ident_bf[:],
            start=(i == 0), stop=(i == 31),
        )

    def flush_pending(pending):
        # one-tile-deferred output drain: by now the PE sum-over-l matmuls
        # for that tile are long done, so ACT never stalls on the PE counter.
        # The weighted sum is over UNNORMALIZED exp weights; the softmax
        # 1/den is applied here on the (otherwise idle) Pool engine so the
        # per-tile DVE stream never waits on the den/recip chain.
        a_prev, rd_prev, tok_prev = pending
        o_sb = op_pool.tile([P, D], FP32, tag="o")
        nc.scalar.copy(o_sb[:], a_prev[:])
        o3 = o_sb[:].rearrange("p (h e) -> p h e", h=H)
        nc.gpsimd.tensor_mul(
            o3, o3, rd_prev[:].unsqueeze(2).broadcast_to([P, H, HD])
        )
        nc.gpsimd.dma_start(out[tok_prev], o_sb[:])

    # ---- software-pipelined loads, one iteration ahead ----
    # ALL DMAs go through the one SWDGE (Pool) ring so their bus order is
    # exactly program order: xt first (tiny; unblocks the PE q_proj that
    # overlaps the k stream), then the big kh cast-DMA, then the previous
    # tile's output store in the slack. Issuing tile i+1's loads at the TOP
    # of iteration i keeps the ring fed so the bus never idles waiting for
    # the Q7 to generate descriptors at an iteration boundary.
    n_iters = NT * repeat
    xt_tiles, k_tiles = {}, {}

    def load_tile(i):
        if i >= n_iters:
            return
        tok_i = slice((i % NT) * P, (i % NT + 1) * P)
        # x arrives pre-arranged from the host as [NT, p, c, t] so each tile
        # is 128 contiguous 4KB rows (one descriptor per partition)
        xs = xtp.tile([P, 8, P], FP32R, tag="xt")
        nc.gpsimd.dma_start(xs[:], xt[i % NT].bitcast(FP32R))
        # kh arrives as bf16 via SWDGE cast-DMA. With SPLIT_K, each tile's
        # load is two 6-layer halves so the scores/exp cascade can start
        # while the second half is still streaming in.
        kb = kp.tile([P, L, D], BF16, tag="k")
        if SPLIT_K or i == 0:
            nc.gpsimd.dma_start(kb[:, 0:6, :], kh[tok_i, 0:6, :])
            nc.gpsimd.dma_start(kb[:, 6:12, :], kh[tok_i, 6:12, :])
        else:
            nc.gpsimd.dma_start(kb[:], kh[tok_i])
        xt_tiles[i] = xs
        k_tiles[i] = kb

    pending = None
    load_tile(0)
    for i in range(n_iters):
        tt = i % NT
        tok = slice(tt * P, (tt + 1) * P)

        load_tile(i + 1)
        xt_sb = xt_tiles.pop(i)
        k_bf = k_tiles.pop(i)
        # The whole q-production chain runs at high priority so the PE order
        # places it ahead of the previous tile's sum-over-l matmuls (the
        # scores-mul waits on q through engine-counter semaphores).
        with tc.high_priority(offset=180):

            # q = x @ W + b (token-major PSUM [t, d_out])
            q_ps = ps_q.tile([P, D], FP32, tag="qps")
            for half in range(2):
                n0 = half * 512
                for c in range(8):
                    nc.tensor.matmul(
                        q_ps[:, n0:n0 + 512],
                        lhsT=xt_sb[:, c, :],
                        rhs=w_sb[:, c, n0:n0 + 512],
                        start=(c == 0),
                        stop=False,
                    )
                nc.tensor.matmul(
                    q_ps[:, n0:n0 + 512],
                    lhsT=ones_sb[:],
                    rhs=bq_sb[:, n0:n0 + 512],
                    start=False,
                    stop=True,
                )
            # q -> SBUF bf16, folding in 1/sqrt(hd), on ACT (keeps DVE free
            # for the scores/weighted-sum muls, which are the busy engine)
            q_bf = qp.tile([P, D], BF16, tag="q")
            nc.scalar.mul(q_bf[:], q_ps[:], SCALE)

        # ---- scores: prod = k * q (broadcast over l), fold-reduce over hd ----
        k4 = k_bf[:].rearrange("p l (h e) -> p l h e", h=H)
        qv = (
            q_bf[:]
            .rearrange("p (h e) -> p h e", h=H)
            .unsqueeze(1)
            .broadcast_to([P, L, H, HD])
        )
        prod = prodp.tile([P, L, H, HD], BF16, tag="prod")
        scr = sp.tile([P, L, H], FP32, tag="scr")
        # exp goes straight to bf16 PAIRS (a pair = one aligned fp32 word)
        # in the expanded-weights tile; den/recip run on the side and only
        # feed the deferred Pool normalize, so the ACT expansion and the
        # DVE weighted-sum muls depend on nothing but the per-half exp.
        wb = wbp.tile([P, L, H, HD], BF16, tag="wb")
        with tc.high_priority(offset=60):
            # two l-halves so compute starts as soon as half of k is in
            halves = (slice(0, 6), slice(6, 12)) if (SPLIT_K or tt == 0) \
                else (slice(0, L),)
            for ls in halves:
                nl = ls.stop - ls.start
                nc.vector.tensor_mul(prod[:, ls], k4[:, ls], qv[:, ls])
                # in-place fold tree over hd: 64->32->...->2, then fp32 tail add.
                # dst aliases in1 exactly (same element positions) which is safe
                # for the streaming DVE.
                off = 0
                for w0 in (32, 16, 8, 4, 2):
                    nc.vector.tensor_add(
                        prod[:, ls, :, off + w0:off + 2 * w0],
                        prod[:, ls, :, off:off + w0],
                        prod[:, ls, :, off + w0:off + 2 * w0],
                    )
                    off += w0
                # off == 62: two surviving partials at 62, 63
                nc.vector.tensor_add(
                    scr[:, ls].unsqueeze(3),
                    prod[:, ls, :, 62:63],
                    prod[:, ls, :, 63:64],
                )
                nc.scalar.activation(
                    wb[:, ls, :, 0:2],
                    scr[:, ls].unsqueeze(3).broadcast_to([P, nl, H, 2]),
                    mybir.ActivationFunctionType.Exp,
                )

        den = sp.tile([P, H], FP32, tag="den")
        nc.vector.tensor_reduce(
            den[:],
            wb[:, :, :, 0].rearrange("p l h -> p h l"),
            axis=mybir.AxisListType.X,
            op=mybir.AluOpType.add,
        )
        rd = sp.tile([P, H], FP32, tag="rd")
        nc.vector.reciprocal(rd[:], den[:])

        # expand each bf16 pair across hd with a step-0-source fp32-word
        # broadcast copy on ACT, in 3 groups of 4 layers pipelined with the
        # weighted-sum muls (fewer, larger DVE ops).
        wbf = wb[:].bitcast(FP32)  # [P, L, H, 32] fp32 words (bf16 pairs)
        wbflat = wb[:].rearrange("p l h e -> p l (h e)")
        acc = ps_a.tile([P, D], FP32, tag="acc")
        for lh in range(6):
            ls = slice(lh * 2, (lh + 1) * 2)
            nc.scalar.copy(
                wbf[:, ls, :, 1:32],
                wbf[:, ls, :, 0:1].broadcast_to([P, 2, H, 31]),
            )
            prod2 = p2p.tile([P, 2, D], BF16, tag="p2")
            nc.vector.tensor_mul(
                prod2[:], k_bf[:, ls, :], wbflat[:, ls, :]
            )
            # sum over l on PE: accumulating identity-matmul copies (fp32
            # PSUM accumulation). prod2 is multi-buffered so the DVE never
            # waits on these within a tile.
            for l in range(ls.start, ls.stop):
                for half in range(2):
                    n0 = half * 512
                    nc.tensor.matmul(
                        acc[:, n0:n0 + 512],
                        lhsT=ident_bf[:],
                        rhs=prod2[:, l - ls.start, n0:n0 + 512],
                        start=(l == 0),
                        stop=(l == L - 1),
                    )
        # flush the PREVIOUS tile here, at the bottom of the iteration: the
        # Pool normalize then runs while this tile's kh stream still owns the
        # DMA bus, instead of sitting between this tile's DMA triggers.
        if pending is not None:
            flush_pending(pending)
        pending = (acc, rd, tok)

    flush_pending(pending)


_NC_CACHE = {}


def build_nc(repeat=1):
    if repeat in _NC_CACHE:
        return _NC_CACHE[repeat]
    nc = bacc.Bacc("TRN2", target_bir_lowering=False, debug=False,
                   num_devices=N_CORES)
    xt = nc.dram_tensor("xt", [NT, P, 8, P], FP32, kind="ExternalInput").ap()
    kh = nc.dram_tensor("kh", [T, L, D], FP32, kind="ExternalInput").ap()
    wq = nc.dram_tensor("wq", [D, D], FP32, kind="ExternalInput").ap()
    bq = nc.dram_tensor("bq", [D], FP32, kind="ExternalInput").ap()
    ones = nc.dram_tensor("ones", [P], FP32, kind="ExternalInput").ap()
    out = nc.dram_tensor("out", [T, D], FP32, kind="ExternalOutput").ap()
    with tile.TileContext(nc) as tc, ExitStack() as ctx:
        build_body(ctx, tc, out, xt, kh, wq, bq, ones, repeat=repeat)
    nc.compile()
    _NC_CACHE[repeat] = nc
    return nc


def make_in_maps(x_current, layer_history, W_q, b_q):
    x_flat = np.ascontiguousarray(
        x_current.reshape(B * S, D), dtype=np.float32)
    k_flat = np.ascontiguousarray(
        layer_history.reshape(B * S, L, D), dtype=np.float32)
    W_q = np.ascontiguousarray(W_q, dtype=np.float32)
    b_q = np.ascontiguousarray(b_q, dtype=np.float32)
    in_maps = []
    for c in range(N_CORES):
        sl = slice(c * T, (c + 1) * T)
        # [NT, p, c, t]: xt[tt, p, ch, t] = x[tt*128 + t, ch*128 + p]
        xt_host = np.ascontiguousarray(
            x_flat[sl].reshape(NT, P, 8, P).transpose(0, 3, 2, 1))
        in_maps.append({
            "xt": xt_host,
            "kh": k_flat[sl],
            "wq": W_q,
            "bq": b_q,
            "ones": np.ones((P,), np.float32),
        })
    return in_maps


def kernel(x_current, layer_history, W_q, b_q):
    nc = build_nc()
    in_maps = make_in_maps(x_current, layer_history, W_q, b_q)
    res = run_bass_kernel_spmd(nc, in_maps, core_ids=list(range(N_CORES)))
    out = np.concatenate([res.results[c]["out"] for c in range(N_CORES)], axis=0)
    return out.reshape(B, S, D).astype(np.float32)


if __name__ == "__main__":
    rng = np.random.default_rng(0)
    x = rng.standard_normal((B, S, D), dtype=np.float32)
    k = rng.standard_normal((B, S, L, D), dtype=np.float32)
    W = (rng.standard_normal((D, D), dtype=np.float32) / math.sqrt(D)).astype(np.float32)
    b = (rng.standard_normal((D,), dtype=np.float32) * 0.01).astype(np.float32)
    o = kernel(x, k, W, b)
    print("ok", o.shape, o.dtype, float(np.abs(o).mean()))



# revision 31
# speedup vs baseline: 1.6849x; 1.0180x over previous
"""Trainium2 Bass kernel for nn_AttentionResidualBlock.

Computation (per token t, head h):
    q = x @ W_q + b_q
    scores[t,h,l] = <q[t,h,:], k[t,l,h,:]> / sqrt(hd)   (k = layer_history)
    w = softmax_l(scores)
    out[t,h,:] = sum_l w[t,h,l] * k[t,l,h,:]

Sharding: data-parallel over the 8192 (b,s) tokens -> 8 cores x 1024 tokens.
Per-core layout: token-major (tokens on partitions), 8 tiles of 128 tokens.

Per tile:
  - layer_history arrives as bf16 via SWDGE cast-DMA (same HBM traffic,
    half the SBUF, and enables the DVE 2x_1P tensor_tensor mode)
  - q_proj on PE in fp32r (full rate at n=512, ~fp32 precision): the x tile
    is transposed with PE transposes, then 16 accumulating matmuls plus a
    k=1 "ones" matmul adds b_q; ACT copies PSUM->SBUF as bf16 with the
    1/sqrt(hd) scale folded in
  - scores: one DVE bf16 mul (q broadcast over l via a step-0 AP dim), then
    an in-place pairwise fold tree over hd (bf16 2x) with an fp32 tail
  - softmax over l=12 without max subtraction (scores ~ N(0,1))
  - normalized weights are written as bf16 pairs and broadcast across hd
    with step-0-source fp32-word copies on ACT, pipelined in 6 groups of 2
    layers with the weighted-sum muls so the serial ACT prefix stays short
  - weighted sum: DVE bf16 mul into a double-buffered product tile; the sum
    over l runs on PE as accumulating identity-matmul copies into PSUM
    (exact fp32 accumulation), with the PSUM->SBUF output drain deferred by
    one tile so ACT never stalls on the PE engine-counter semaphore
DVE is the bottleneck engine (~22 us/tile busy); DMA ~21 us/tile, PE ~17,
ACT ~11. Measured ~193 us/core on HW median (DMA roofline ~165 us).
"""

import math
from contextlib import ExitStack

import numpy as np

import concourse.tile as tile
from concourse import bacc, mybir
from concourse.bass_utils import run_bass_kernel_spmd
from concourse import masks

FP32 = mybir.dt.float32
FP32R = mybir.dt.float32r
BF16 = mybir.dt.bfloat16

B, S, L, D, H = 4, 2048, 12, 1024, 16
HD = D // H
N_CORES = 8
T = B * S // N_CORES          # tokens per core = 1024
P = 128                       # partition tile
NT = T // P                   # 8 token tiles per core
SCALE = 1.0 / math.sqrt(HD)   # 0.125
SPLIT_K = False                # per-tile kh DMA + scores cascade in 2 halves


def build_body(ctx, tc, out, xt, kh, wq, bq, ones, repeat=1):
    nc = tc.nc

    const_pool = ctx.enter_context(tc.tile_pool(name="const", bufs=1))
    # W as lhsT chunks: w_sb[p, c, j] = W[c*128 + p, j]
    w_sb = const_pool.tile([P, 8, D], FP32R)
    wqr = wq.rearrange("(c p) j -> p c j", p=P).bitcast(FP32R)
    nc.scalar.dma_start(w_sb[:, :, 0:512], wqr[:, :, 0:512])
    nc.scalar.dma_start(w_sb[:, :, 512:1024], wqr[:, :, 512:1024])
    bq_sb = const_pool.tile([1, D], BF16)
    nc.gpsimd.dma_start(bq_sb[:], bq.unsqueeze(0))
    ones_sb = const_pool.tile([1, P], BF16)
    nc.gpsimd.dma_start(ones_sb[:], ones.unsqueeze(0))
    ident_bf = const_pool.tile([P, P], BF16)
    masks.make_identity(nc, ident_bf[:])

    kp = ctx.enter_context(tc.tile_pool(name="k", bufs=3))
    xtp = ctx.enter_context(tc.tile_pool(name="xt", bufs=3))
    qp = ctx.enter_context(tc.tile_pool(name="q", bufs=3))
    prodp = ctx.enter_context(tc.tile_pool(name="prod", bufs=1))
    p2p = ctx.enter_context(tc.tile_pool(name="p2", bufs=3))
    wbp = ctx.enter_context(tc.tile_pool(name="wb", bufs=1))
    op_pool = ctx.enter_context(tc.tile_pool(name="osb", bufs=2))
    sp = ctx.enter_context(tc.tile_pool(name="smx", bufs=2))
    ps_t = ctx.enter_context(tc.tile_pool(name="ps_t", bufs=2, space="PSUM"))
    ps_q = ctx.enter_context(tc.tile_pool(name="ps_q", bufs=1, space="PSUM"))
    ps_a = ctx.enter_context(tc.tile_pool(name="ps_a", bufs=2, space="PSUM"))

    # PE warm-up: ~4us of dummy matmuls at t~0 so the HAM clock-gate opens
    # before tile 0's q_proj (cold PE is the prologue critical path)
    warm_ps = ps_t.tile([P, P], FP32, tag="xtps")
    for i in range(32):
        nc.tensor.matmul(
            warm_ps[:], lhsT=ident_bf[:], rhs=ident_bf[:],
            start=(i == 0), stop=(i == 31),
        )

    def flush_pending(pending):
        # one-tile-deferred output drain: by now the PE sum-over-l matmuls
        # for that tile are long done, so ACT never stalls on the PE counter.
        # The weighted sum is over UNNORMALIZED exp weights; the softmax
        # 1/den is applied here on the (otherwise idle) Pool engine so the
        # per-tile DVE stream never waits on the den/recip chain.
        a_prev, rd_prev, tok_prev = pending
        o_sb = op_pool.tile([P, D], FP32, tag="o")
        nc.scalar.copy(o_sb[:], a_prev[:])
        o3 = o_sb[:].rearrange("p (h e) -> p h e", h=H)
        nc.gpsimd.tensor_mul(
            o3, o3, rd_prev[:].unsqueeze(2).broadcast_to([P, H, HD])
        )
        nc.gpsimd.dma_start(out[tok_prev], o_sb[:])

    # ---- software-pipelined loads, one iteration ahead ----
    # ALL DMAs go through the one SWDGE (Pool) ring so their bus order is
    # exactly program order: xt first (tiny; unblocks the PE q_proj that
    # overlaps the k stream), then the big kh cast-DMA, then the previous
    # tile's output store in the slack. Issuing tile i+1's loads at the TOP
    # of iteration i keeps the ring fed so the bus never idles waiting for
    # the Q7 to generate descriptors at an iteration boundary.
    n_iters = NT * repeat
    xt_tiles, k_tiles = {}, {}

    def load_tile(i):
        if i >= n_iters:
            return
        tok_i = slice((i % NT) * P, (i % NT + 1) * P)
        # x arrives pre-arranged from the host as [NT, p, c, t] so each tile
        # is 128 contiguous 4KB rows (one descriptor per partition)
        xs = xtp.tile([P, 8, P], FP32R, tag="xt")
        nc.gpsimd.dma_start(xs[:], xt[i % NT].bitcast(FP32R))
        # kh arrives as bf16 via SWDGE cast-DMA. With SPLIT_K, each tile's
        # load is two 6-layer halves so the scores/exp cascade can start
        # while the second half is still streaming in.
        kb = kp.tile([P, L, D], BF16, tag="k")
        if SPLIT_K or i == 0:
            nc.gpsimd.dma_start(kb[:, 0:6, :], kh[tok_i, 0:6, :])
            nc.gpsimd.dma_start(kb[:, 6:12, :], kh[tok_i, 6:12, :])
        else:
            nc.gpsimd.dma_start(kb[:], kh[tok_i])
        xt_tiles[i] = xs
        k_tiles[i] = kb

    pending = None
    load_tile(0)
    for i in range(n_iters):
        tt = i % NT
        tok = slice(tt * P, (tt + 1) * P)

        load_tile(i + 1)
        xt_sb = xt_tiles.pop(i)
        k_bf = k_tiles.pop(i)
        # The whole q-production chain runs at high priority so the PE order
        # places it ahead of the previous tile's sum-over-l matmuls (the
        # scores-mul waits on q through engine-counter semaphores).
        with tc.high_priority(offset=180):

            # q = x @ W + b (token-major PSUM [t, d_out])
            q_ps = ps_q.tile([P, D], FP32, tag="qps")
            for half in range(2):
                n0 = half * 512
                for c in range(8):
                    nc.tensor.matmul(
                        q_ps[:, n0:n0 + 512],
                        lhsT=xt_sb[:, c, :],
                        rhs=w_sb[:, c, n0:n0 + 512],
                        start=(c == 0),
                        stop=False,
                    )
                nc.tensor.matmul(
                    q_ps[:, n0:n0 + 512],
                    lhsT=ones_sb[:],
                    rhs=bq_sb[:, n0:n0 + 512],
                    start=False,
                    stop=True,
                )
            # q -> SBUF bf16, folding in 1/sqrt(hd), on ACT (keeps DVE free
            # for the scores/weighted-sum muls, which are the busy engine)
            q_bf = qp.tile([P, D], BF16, tag="q")
            nc.scalar.mul(q_bf[:], q_ps[:], SCALE)

        # ---- scores: prod = k * q (broadcast over l), fold-reduce over hd ----
        k4 = k_bf[:].rearrange("p l (h e) -> p l h e", h=H)
        qv = (
            q_bf[:]
            .rearrange("p (h e) -> p h e", h=H)
            .unsqueeze(1)
            .broadcast_to([P, L, H, HD])
        )
        prod = prodp.tile([P, L, H, HD], BF16, tag="prod")
        scr = sp.tile([P, L, H], FP32, tag="scr")
        # exp goes straight to bf16 PAIRS (a pair = one aligned fp32 word)
        # in the expanded-weights tile; den/recip run on the side and only
        # feed the deferred Pool normalize, so the ACT expansion and the
        # DVE weighted-sum muls depend on nothing but the per-half exp.
        wb = wbp.tile([P, L, H, HD], BF16, tag="wb")
        with tc.high_priority(offset=60):
            # two l-halves so compute starts as soon as half of k is in
            halves = (slice(0, 6), slice(6, 12)) if (SPLIT_K or tt == 0) \
                else (slice(0, L),)
            for ls in halves:
                nl = ls.stop - ls.start
                nc.vector.tensor_mul(prod[:, ls], k4[:, ls], qv[:, ls])
                # in-place fold tree over hd: 64->32->...->2, then fp32 tail add.
                # dst aliases in1 exactly (same element positions) which is safe
                # for the streaming DVE.
                off = 0
                for w0 in (32, 16, 8, 4, 2):
                    nc.vector.tensor_add(
                        prod[:, ls, :, off + w0:off + 2 * w0],
                        prod[:, ls, :, off:off + w0],
                        prod[:, ls, :, off + w0:off + 2 * w0],
                    )
                    off += w0
                # off == 62: two surviving partials at 62, 63
                nc.vector.tensor_add(
                    scr[:, ls].unsqueeze(3),
                    prod[:, ls, :, 62:63],
                    prod[:, ls, :, 63:64],
                )
                nc.scalar.activation(
                    wb[:, ls, :, 0:2],
                    scr[:, ls].unsqueeze(3).broadcast_to([P, nl, H, 2]),
                    mybir.ActivationFunctionType.Exp,
                )

        den = sp.tile([P, H], FP32, tag="den")
        nc.vector.tensor_reduce(
            den[:],
            wb[:, :, :, 0].rearrange("p l h -> p h l"),
            axis=mybir.AxisListType.X,
            op=mybir.AluOpType.add,
        )
        rd = sp.tile([P, H], FP32, tag="rd")
        nc.vector.reciprocal(rd[:], den[:])

        # expand each bf16 pair across hd with a step-0-source fp32-word
        # broadcast copy on ACT, in 3 groups of 4 layers pipelined with the
        # weighted-sum muls (fewer, larger DVE ops).
        wbf = wb[:].bitcast(FP32)  # [P, L, H, 32] fp32 words (bf16 pairs)
        wbflat = wb[:].rearrange("p l h e -> p l (h e)")
        acc = ps_a.tile([P, D], FP32, tag="acc")
        for lh in range(6):
            ls = slice(lh * 2, (lh + 1) * 2)
            nc.scalar.copy(
                wbf[:, ls, :, 1:32],
                wbf[:, ls, :, 0:1].broadcast_to([P, 2, H, 31]),
            )
            prod2 = p2p.tile([P, 2, D], BF16, tag="p2")
            nc.vector.tensor_mul(
                prod2[:], k_bf[:, ls, :], wbflat[:, ls, :]
            )
            # sum over l on PE: accumulating identity-matmul copies (fp32
            # PSUM accumulation). prod2 is multi-buffered so the DVE never
            # waits on these within a tile.
            for l in range(ls.start, ls.stop):
                for half in range(2):
                    n0 = half * 512
                    nc.tensor.matmul(
                        acc[:, n0:n0 + 512],
                        lhsT=ident_bf[:],
                        rhs=prod2[:, l - ls.start, n0:n0 + 512],
                        start=(l == 0),
                        stop=(l == L - 1),
                    )
        # flush the PREVIOUS tile here, at the bottom of the iteration: the
        # Pool normalize then runs while this tile's kh stream still owns the
        # DMA bus, instead of sitting between this tile's DMA triggers.
        if pending is not None:
            flush_pending(pending)
        pending = (acc, rd, tok)

    flush_pending(pending)


_NC_CACHE = {}


def build_nc(repeat=1):
    if repeat in _NC_CACHE:
        return _NC_CACHE[repeat]
    nc = bacc.Bacc("TRN2", target_bir_lowering=False, debug=False,
                   num_devices=N_CORES)
    xt = nc.dram_tensor("xt", [NT, P, 8, P], FP32, kind="ExternalInput").ap()
    kh = nc.dram_tensor("kh", [T, L, D], FP32, kind="ExternalInput").ap()
    wq = nc.dram_tensor("wq", [D, D], FP32, kind="ExternalInput").ap()
    bq = nc.dram_tensor("bq", [D], FP32, kind="ExternalInput").ap()
    ones = nc.dram_tensor("ones", [P], FP32, kind="ExternalInput").ap()
    out = nc.dram_tensor("out", [T, D], FP32, kind="ExternalOutput").ap()
    with tile.TileContext(nc) as tc, ExitStack() as ctx:
        build_body(ctx, tc, out, xt, kh, wq, bq, ones, repeat=repeat)
    nc.compile()
    _NC_CACHE[repeat] = nc
    return nc


def make_in_maps(x_current, layer_history, W_q, b_q):
    x_flat = np.ascontiguousarray(
        x_current.reshape(B * S, D), dtype=np.float32)
    k_flat = np.ascontiguousarray(
        layer_history.reshape(B * S, L, D), dtype=np.float32)
    W_q = np.ascontiguousarray(W_q, dtype=np.float32)
    b_q = np.ascontiguousarray(b_q, dtype=np.float32)
    in_maps = []
    for c in range(N_CORES):
        sl = slice(c * T, (c + 1) * T)
        # [NT, p, c, t]: xt[tt, p, ch, t] = x[tt*128 + t, ch*128 + p]
        xt_host = np.ascontiguousarray(
            x_flat[sl].reshape(NT, P, 8, P).transpose(0, 3, 2, 1))
        in_maps.append({
            "xt": xt_host,
            "kh": k_flat[sl],
            "wq": W_q,
            "bq": b_q,
            "ones": np.ones((P,), np.float32),
        })
    return in_maps


def kernel(x_current, layer_history, W_q, b_q):
    nc = build_nc()
    in_maps = make_in_maps(x_current, layer_history, W_q, b_q)
    res = run_bass_kernel_spmd(nc, in_maps, core_ids=list(range(N_CORES)))
    out = np.concatenate([res.results[c]["out"] for c in range(N_CORES)], axis=0)
    return out.reshape(B, S, D).astype(np.float32)


if __name__ == "__main__":
    rng = np.random.default_rng(0)
    x = rng.standard_normal((B, S, D), dtype=np.float32)
    k = rng.standard_normal((B, S, L, D), dtype=np.float32)
    W = (rng.standard_normal((D, D), dtype=np.float32) / math.sqrt(D)).astype(np.float32)
    b = (rng.standard_normal((D,), dtype=np.float32) * 0.01).astype(np.float32)
    o = kernel(x, k, W, b)
    print("ok", o.shape, o.dtype, float(np.abs(o).mean()))

